# revision 38
# baseline (speedup 1.0000x reference)
"""Multi-head attention kernel for 8 Trainium2 NeuronCores.

Problem: x[4, 2048, 1024], 16 heads x 64 head-dim MHA (QKV proj -> softmax
attention -> out proj), fp32.

Sharding: 8 cores = 4 batches x 2 head-groups. Core c handles batch c//2 and
heads (c%2)*8 .. (c%2)*8+7 (tensor-parallel split of the QKV/out projections).
Each core computes a partial output [2048, 1024] (its 8 heads through Wo);
the host sums the two partials per batch and adds bo.

Per-core kernel (all matmuls in fp32r = full-rate TF32-like):
  A. transpose x via PE -> xT [dim, seq]
  B. QKV: Q^T,K^T [hd, seq] (K^T scaled by 1/8), V in natural [seq, hd] layout
     augmented with a ones column per head (Vaug) for the softmax denominator
  C. per head-pair, per 512-wide q-chunk, loop over 16 k-tiles:
     scores S^T[k,q] = K^T.T @ Q^T for both heads (row-packed, K=64 each),
     exp on ScalarE (no max subtraction needed; |scores| ~ few units),
     U^T[hd+1, q] += Vaug.T @ exp(S^T)  (row 64 = softmax denominator r),
     then C^T = U^T * (1/r) broadcast via DMA, stored to UT tiles
  D. out = C @ Wo per seq-tile (contraction over all 512 local hd dims)
"""

import numpy as np

B = 4
SEQ = 2048
DIM = 1024
NH_LOC = 8      # heads per core
HID = 64
HDL = NH_LOC * HID  # 512
N_CORES = 8

_PROG = None


def _build_program(seq=SEQ, reps=1):
    import contextlib

    import concourse.bass as bass
    import concourse.mybir as mybir
    import concourse.tile as tile
    from concourse import bacc
    from concourse.masks import make_identity

    FP32 = mybir.dt.float32
    FP32R = mybir.dt.float32r
    Exp = mybir.ActivationFunctionType.Exp
    Alu = mybir.AluOpType

    seq_t = seq // 128            # seq tiles
    dim_t = DIM // 128            # 8
    n_half = 2 if seq >= 1024 else 1
    sh_seq = seq // n_half        # seq cols per half
    sh_t = seq_t // n_half        # seq tiles per half
    sh_c = sh_seq // 512          # 512-chunks per half
    n_qc = seq // 512             # q chunks
    n_hp = NH_LOC // 2            # head pairs = 4
    n_m = HDL // 128              # hd-row tiles = 4

    nc = bacc.Bacc()
    x_d = nc.declare_dram_parameter("x", [seq, DIM], FP32, isOutput=False)
    wq_d = nc.declare_dram_parameter("wq", [DIM, HDL], FP32, isOutput=False)
    wk_d = nc.declare_dram_parameter("wk", [DIM, HDL], FP32, isOutput=False)
    wv_d = nc.declare_dram_parameter("wv", [DIM, HDL], FP32, isOutput=False)
    bq_d = nc.declare_dram_parameter("bq", [HDL], FP32, isOutput=False)
    bk_d = nc.declare_dram_parameter("bk", [HDL], FP32, isOutput=False)
    bv_d = nc.declare_dram_parameter("bv", [HDL], FP32, isOutput=False)
    wo_d = nc.declare_dram_parameter("wo", [HDL, DIM], FP32, isOutput=False)
    out_d = nc.declare_dram_parameter("out", [seq, DIM], FP32, isOutput=True)
    rrs_d = nc.dram_tensor("rrs", [NH_LOC // 2, seq // 512, 2, 512], FP32)

    with tile.TileContext(nc, pool_alloc_mode="queue") as tc:
        with (
            tc.tile_pool(name="persist", bufs=1) as persist,
        ):
            ident = persist.tile([128, 128], FP32)
            make_identity(nc, ident[:])

            QT = [[persist.tile([128, sh_seq], FP32R, tag=f"qt{m}_{h}", name=f"qt{m}_{h}")
                   for h in range(n_half)] for m in range(n_m)]
            KT = [[persist.tile([128, sh_seq], FP32R, tag=f"kt{m}_{h}", name=f"kt{m}_{h}")
                   for h in range(n_half)] for m in range(n_m)]
            Vaug = [persist.tile([128, NH_LOC * (HID + 1)], FP32R, tag=f"va{st}", name=f"va{st}")
                    for st in range(seq_t)]

            bq_sb = persist.tile([128, n_m], FP32)
            bk_sb = persist.tile([128, n_m], FP32)
            nc.sync.dma_start(out=bq_sb[:], in_=bq_d[:].rearrange("(m p) -> p m", p=128))
            nc.sync.dma_start(out=bk_sb[:], in_=bk_d[:].rearrange("(m p) -> p m", p=128))
            bv_bc = persist.tile([128, HDL], FP32)
            bv_ap = bv_d[:]
            nc.sync.dma_start(
                out=bv_bc[:],
                in_=bass.AP(tensor=bv_ap.tensor, offset=bv_ap.offset,
                            ap=[[0, 128], [1, HDL]]),
            )
            ones8 = persist.tile([128, NH_LOC], FP32)
            nc.vector.memset(ones8[:], 1.0)

            rep_ctx = tc.For_i(0, reps, 1) if reps > 1 else contextlib.nullcontext()
            with rep_ctx:
                _build_body(nc, tc, bass, mybir, tile, make_identity, locals())

    nc.compile()
    return nc


def _build_body(nc, tc, bass, mybir, tile, make_identity, env):
    FP32 = mybir.dt.float32
    FP32R = mybir.dt.float32r
    Exp = mybir.ActivationFunctionType.Exp
    Alu = mybir.AluOpType
    seq = env["seq"]
    seq_t, dim_t, n_half = env["seq_t"], env["dim_t"], env["n_half"]
    sh_seq, sh_t, sh_c = env["sh_seq"], env["sh_t"], env["sh_c"]
    n_qc, n_hp, n_m = env["n_qc"], env["n_hp"], env["n_m"]
    persist = env["persist"]
    ident, QT, KT, Vaug = env["ident"], env["QT"], env["KT"], env["Vaug"]
    bq_sb, bk_sb, bv_bc, ones8 = env["bq_sb"], env["bk_sb"], env["bv_bc"], env["ones8"]
    x_d, wq_d, wk_d, wv_d = env["x_d"], env["wq_d"], env["wk_d"], env["wv_d"]
    wo_d, out_d, rrs_d = env["wo_d"], env["out_d"], env["rrs_d"]

    if True:
            # ---------- Phase A+B: transpose x, QKV projections ----------
            with (
                tc.tile_pool(name="xstage", bufs=3) as xstage,
                tc.tile_pool(name="xtp", bufs=1) as xtp,
                tc.tile_pool(name="wstage", bufs=3) as wstage,
                tc.tile_pool(name="wpool", bufs=2) as wpool,
                tc.tile_pool(name="tpp", bufs=2, space="PSUM") as tpp,
                tc.tile_pool(name="qkvp", bufs=4, space="PSUM") as qkvp,
            ):
                for sh in range(n_half):
                    # xTall column layout: [dim-tile d][seq col] (d-major)
                    xTall = xtp.tile([128, dim_t * sh_seq], FP32R, tag="xtall",
                                     name="xtall")
                    xT = [xTall[:, d*sh_seq:(d+1)*sh_seq] for d in range(dim_t)]
                    for st8 in range(sh_t):
                        st = sh * sh_t + st8
                        xst = xstage.tile([128, DIM], FP32, tag="xst")
                        dma_eng = nc.sync if st % 2 == 0 else nc.scalar
                        dma_eng.dma_start(out=xst[:], in_=x_d[st*128:(st+1)*128, :])
                        for dg in range(dim_t // 4):
                            tp = tpp.tile([128, 512], FP32, tag="tp")
                            for j in range(4):
                                d = dg * 4 + j
                                nc.tensor.transpose(
                                    tp[:, j*128:(j+1)*128],
                                    xst[:, d*128:(d+1)*128], ident[:])
                            # one strided copy: 4 transposed blocks -> 4 xT tiles
                            out_ap = xTall[:].rearrange(
                                "p (d s) -> p d s", d=dim_t)[
                                :, dg*4:(dg+1)*4, st8*128:(st8+1)*128]
                            nc.vector.tensor_copy(
                                out_ap,
                                tp[:].rearrange("p (j c) -> p j c", c=128))

                    for proj, w_dram, dst, bias, scale in (
                        ("k", wk_d, KT, bk_sb, 0.125),
                        ("v", wv_d, None, None, None),
                        ("q", wq_d, QT, bq_sb, None),
                    ):
                        wr = []
                        for d in range(dim_t):
                            wst = wstage.tile([128, HDL], FP32, tag="wst")
                            nc.sync.dma_start(out=wst[:], in_=w_dram[d*128:(d+1)*128, :])
                            wrd = wpool.tile([128, HDL], FP32R, tag=f"w{d}")
                            nc.vector.tensor_copy(wrd[:], wst[:])
                            wr.append(wrd)
                        if proj != "v":
                            # dst[m][:, cols] = (x @ W + b)^T (scaled for K)
                            for m in range(n_m):
                                for sc in range(sh_c):
                                    qp = qkvp.tile([128, 512], FP32, tag="qp")
                                    for d in range(dim_t):
                                        nc.tensor.matmul(
                                            qp[:],
                                            wr[d][:, m*128:(m+1)*128],
                                            xT[d][:, sc*512:(sc+1)*512],
                                            start=(d == 0), stop=(d == dim_t - 1),
                                        )
                                    col0 = sc * 512
                                    if scale is None:
                                        nc.vector.tensor_scalar(
                                            dst[m][sh][:, col0:col0+512], qp[:],
                                            bias[:, m:m+1], None, Alu.add)
                                    else:
                                        nc.vector.tensor_scalar(
                                            dst[m][sh][:, col0:col0+512], qp[:],
                                            bias[:, m:m+1], scale, Alu.add, Alu.mult)
                        else:
                            # V natural [seq, hd] + bias, strided into Vaug
                            for st8 in range(sh_t):
                                st = sh * sh_t + st8
                                vp = qkvp.tile([128, HDL], FP32, tag="qp")
                                for d in range(dim_t):
                                    nc.tensor.matmul(
                                        vp[:],
                                        xT[d][:, st8*128:(st8+1)*128],
                                        wr[d][:],
                                        start=(d == 0), stop=(d == dim_t - 1),
                                    )
                                va3 = Vaug[st][:].rearrange("p (h c) -> p h c", c=HID+1)
                                nc.vector.tensor_tensor(
                                    va3[:, :, 0:HID],
                                    vp[:].rearrange("p (h c) -> p h c", c=HID),
                                    bv_bc[:].rearrange("p (h c) -> p h c", c=HID),
                                    Alu.add)
                                nc.vector.tensor_copy(
                                    va3[:, :, HID:HID+1],
                                    ones8[:].rearrange("p (h c) -> p h c", c=1))

            # ---------- Phase C+D: attention + output projection ----------
            # Two independent (hp, qc) streams are interleaved per kt step so
            # ScalarE (exp, the critical resource) never waits on the PE
            # scores->attnV chain of a single stream. The output projection
            # for a q-chunk pair runs as soon as all head-pairs finished it,
            # hiding phase D under the next chunk's attention.
            utpool_cm = tc.tile_pool(name="utpool", bufs=1)
            utpool = utpool_cm.__enter__()
            UT = [[utpool.tile([128, 512], FP32R, tag=f"ut{hp}_{q}", name=f"ut{hp}_{q}")
                   for q in range(n_qc)] for hp in range(n_hp)]
            with (
                tc.tile_pool(name="epool", bufs=6) as epool,
                tc.tile_pool(name="rpool", bufs=4) as rpool,
                tc.tile_pool(name="rbpool", bufs=6) as rbpool,
                tc.tile_pool(name="sps", bufs=2, space="PSUM") as sps,
                tc.tile_pool(name="ups", bufs=1, space="PSUM") as ups,
            ):
                def attn_stream(sid, hp, qc):
                    """Emit one (hp, qc) attention unit using stream slot sid."""
                    vca = 2 * hp * (HID + 1)
                    vcb = (2 * hp + 1) * (HID + 1)
                    ua = ups.tile([HID + 1, 512], FP32, tag=f"ua{sid}",
                                  name=f"ua{sid}")
                    ub = ups.tile([HID + 1, 512], FP32, tag=f"ub{sid}",
                                  name=f"ub{sid}")
                    # scores+exp emitted at kt; the dependent attnV matmuls
                    # are emitted one kt later so the in-order PE queue never
                    # head-of-line blocks waiting for the exp.
                    steps = []
                    attns = []
                    for kt in range(seq_t):
                        def step(kt=kt, hp=hp, qc=qc, vca=vca, vcb=vcb,
                                 ua=ua, ub=ub):
                            s2 = sps.tile([128, 1024], FP32, tag="s2", name="s2")
                            kth = KT[hp][kt // sh_t]
                            kc0 = (kt % sh_t) * 128
                            qth = QT[hp][(qc * 512) // sh_seq]
                            qc0 = (qc * 512) % sh_seq
                            nc.tensor.matmul(
                                s2[:, 0:512],
                                kth[0:64, kc0:kc0+128],
                                qth[0:64, qc0:qc0+512],
                                start=True, stop=True)
                            nc.tensor.matmul(
                                s2[:, 512:1024],
                                kth[64:128, kc0:kc0+128],
                                qth[64:128, qc0:qc0+512],
                                start=True, stop=True)
                            e2 = epool.tile([128, 1024], FP32R, tag="e2", name="e2")
                            nc.scalar.activation(e2[:], s2[:], Exp)
                            def attn(e2=e2, kt=kt, ua=ua, ub=ub,
                                     vca=vca, vcb=vcb):
                                nc.tensor.matmul(
                                    ua[:], Vaug[kt][:, vca:vca+HID+1],
                                    e2[:, 0:512],
                                    start=(kt == 0), stop=(kt == seq_t - 1))
                                nc.tensor.matmul(
                                    ub[:], Vaug[kt][:, vcb:vcb+HID+1],
                                    e2[:, 512:1024],
                                    start=(kt == 0), stop=(kt == seq_t - 1))
                            attns.append(attn)
                        steps.append(step)

                    def finish(ua=ua, ub=ub, hp=hp, qc=qc):
                        for hi, (ui, rowbase) in enumerate(((ua, 0), (ub, 64))):
                            # copy U psum->sbuf first so the psum bank frees
                            # quickly; normalize from the sbuf copy.
                            usb = rbpool.tile([HID + 1, 512], FP32, tag="usb")
                            nc.vector.tensor_copy(usb[:], ui[:])
                            rr = rpool.tile([1, 512], FP32, tag="rr")
                            nc.vector.reciprocal(rr[:], usb[HID:HID+1, :])
                            slot = rrs_d[hp, qc, hi, :]
                            nc.sync.dma_start(out=slot, in_=rr[0:1, :])
                            rb = rbpool.tile([HID, 512], FP32, tag="rb")
                            nc.sync.dma_start(
                                out=rb[:],
                                in_=bass.AP(tensor=slot.tensor,
                                            offset=slot.offset,
                                            ap=[[0, HID], [1, 512]]))
                            nc.vector.tensor_tensor(
                                UT[hp][qc][rowbase:rowbase+HID, :],
                                usb[0:HID, :], rb[:], Alu.mult)
                    return steps, attns, finish

                # (hp, qc) units in qc-major order; run two streams at a time.
                units = [(hp, qc) for qc in range(n_qc) for hp in range(n_hp)]
                for u in range(0, len(units), 2):
                    pair = units[u:u+2]
                    streams = [attn_stream(i, hp, qc)
                               for i, (hp, qc) in enumerate(pair)]
                    for kt in range(seq_t):
                        for stream in streams:
                            stream[0][kt]()      # scores + exp
                        if kt > 0:
                            for stream in streams:
                                stream[1][kt - 1]()  # attnV of previous kt
                    for stream in streams:
                        stream[1][seq_t - 1]()
                    for stream in streams:
                        stream[2]()

            # ---------- Phase D: output projection ----------
            with (
                tc.tile_pool(name="wostage", bufs=2) as wostage,
                tc.tile_pool(name="wopool", bufs=1) as wopool,
                tc.tile_pool(name="outstage", bufs=3) as outstage,
                tc.tile_pool(name="ops", bufs=4, space="PSUM") as ops,
            ):
                wo_r = []
                for hp in range(n_hp):
                    wos = wostage.tile([128, DIM], FP32, tag="wos")
                    nc.sync.dma_start(out=wos[:], in_=wo_d[hp*128:(hp+1)*128, :])
                    wr = wopool.tile([128, DIM], FP32R, tag=f"wo{hp}", name=f"wo{hp}")
                    nc.vector.tensor_copy(wr[:], wos[:])
                    wo_r.append(wr)
                for st in range(seq_t):
                    ot = outstage.tile([128, DIM], FP32, tag="ot")
                    for oc in range(DIM // 512):
                        op_t = ops.tile([128, 512], FP32, tag="op")
                        for hp in range(n_hp):
                            nc.tensor.matmul(
                                op_t[:],
                                UT[hp][st // 4][:, (st % 4)*128:(st % 4)*128+128],
                                wo_r[hp][:, oc*512:(oc+1)*512],
                                start=(hp == 0), stop=(hp == n_hp - 1))
                        # split psum->sbuf copies between DVE and idle ScalarE
                        if oc == 0:
                            nc.vector.tensor_copy(ot[:, oc*512:(oc+1)*512],
                                                  op_t[:])
                        else:
                            nc.scalar.copy(ot[:, oc*512:(oc+1)*512], op_t[:])
                    (nc.sync if st % 2 == 0 else nc.scalar).dma_start(
                        out=out_d[st*128:(st+1)*128, :], in_=ot[:])
            utpool_cm.__exit__(None, None, None)


def _get_program():
    global _PROG
    if _PROG is None:
        _PROG = _build_program()
    return _PROG


def _make_in_maps(inputs):
    x = np.asarray(inputs["x"], dtype=np.float32)
    in_maps = []
    for c in range(N_CORES):
        b, g = divmod(c, 2)
        sl = slice(g * HDL, (g + 1) * HDL)
        in_maps.append({
            "x": np.ascontiguousarray(x[b]),
            "wq": np.ascontiguousarray(np.asarray(inputs["Wq"], np.float32)[:, sl]),
            "wk": np.ascontiguousarray(np.asarray(inputs["Wk"], np.float32)[:, sl]),
            "wv": np.ascontiguousarray(np.asarray(inputs["Wv"], np.float32)[:, sl]),
            "bq": np.ascontiguousarray(np.asarray(inputs["bq"], np.float32)[sl]),
            "bk": np.ascontiguousarray(np.asarray(inputs["bk"], np.float32)[sl]),
            "bv": np.ascontiguousarray(np.asarray(inputs["bv"], np.float32)[sl]),
            "wo": np.ascontiguousarray(np.asarray(inputs["Wo"], np.float32)[sl, :]),
        })
    return in_maps


def kernel(x, Wq, bq, Wk, bk, Wv, bv, Wo, bo):
    from concourse.bass_utils import run_bass_kernel_spmd

    bo = np.asarray(bo, dtype=np.float32)
    nc = _get_program()
    in_maps = _make_in_maps(dict(x=x, Wq=Wq, bq=bq, Wk=Wk, bk=bk, Wv=Wv, bv=bv,
                                 Wo=Wo, bo=bo))
    res = run_bass_kernel_spmd(nc, in_maps, core_ids=list(range(N_CORES)))
    out = np.empty((B, SEQ, DIM), dtype=np.float32)
    for b in range(B):
        out[b] = res.results[2 * b]["out"] + res.results[2 * b + 1]["out"] + bo
    return out
